# revision 51
# baseline (speedup 1.0000x reference)
"""Trainium2 Bass kernel for nn_CubECLayr: Euler characteristic curves of
sublevel cubical complexes, batch-data-parallel over 8 NeuronCores.

Algorithm (per core, 24 images of 256x256):
  1. kt = 2*ceil(x/DT) per pixel (exact even-integer bin, via fused multiply
     + magic-number round).  kt in [2, 62] for x in (0,1).
  2. Vertex attribution: every cell (vertex/edge/square) of the cubical
     complex is anchored to its (value, index)-max vertex; the signed count
     of cells anchored at each pixel is an integer delta computed from
     neighbor comparisons in kt-space.  Then ECC_s = sum_p delta_p*[k_p <= s].
  3. Per-bin fields F_b = sum_p delta_p*[k_p == b] are computed two ways,
     split across engines:
       - bins 1..24 on the Vector engine: a custom DVE op accumulates TWO
         radix-packed bins per pass (accum += delta*([kt==2b0]+2048*[kt==2b1]),
         12 passes; fields bounded by 161 << 1024 so fp32 decodes exactly).
       - bins 25..30 on the otherwise-idle Scalar engine via relu moments:
         km = kt + (delta+8)/16;  R_c = sum_p relu(km - c) for c = 49..62
         (14 activation passes with accumulator).  Host-side second
         differences give tents T_b = R_{2b}-2R_{2b+1}+R_{2b+2} and
         U_b = R_{2b-1}-2R_{2b}+R_{2b+1} with F_b = 8*(T_b - U_b), exact in
         fp32 (all values are multiples of 1/16 bounded by 2^23).
     bin 31 = tot - sum(bins 1..30) on host, where tot = sum(delta) rides
     the accumulator of a Scalar copy of delta.
  4. Per-partition partials reduce to per-image values by two small PE
     matmuls into one PSUM tile ([33+14, 24]); host does the final cumsum.

Pipelining: kt is double-buffered; chunk c+1's DMA + Scalar bin-conversion
is issued between chunk c's delta assembly and its histogram passes.  The
Scalar relu passes for chunk c run concurrently with chunk c's (and early
chunk c+1's) Vector histogram passes.

Layout: 3 chunks x 8 images; each image owns 16 partitions (16 rows each,
one halo row above/below; image-boundary halo rows are PAD).
"""

from operator import add as _operator_add

import numpy as np

import concourse.bacc as bacc
import concourse.mybir as mybir
from concourse import tile
from concourse.bass_utils import run_bass_kernel_spmd

NCORES = 8
B, C, H, W = 64, 3, 256, 256
IMGS = (B // NCORES) * C          # 24 images per core
CHUNK_IMGS = 8
NCHUNK = IMGS // CHUNK_IMGS       # 3
RB = 16                           # partitions per image
ROWS = H // RB                    # 16 own rows per partition
FD = ROWS * W                     # 4096 own pixels per partition
STEPS = 32
DVEBINS = 22                      # bins 1..22 via custom DVE passes
NH = DVEBINS + 1                  # hist cols: 0 = tot, 1..24 = DVE bins
RBASE = 32                        # R rows base partition in PSUM (HW: 0/32/64)
NPAIR = DVEBINS // 2              # 12 packed passes
TENT_LO = DVEBINS + 1             # bins 25..30 via Scalar relu moments
TENT_HI = 30
NRC = 2 * TENT_HI + 2 - (2 * TENT_LO - 1) + 1   # R_c count: c = 49..62
RC0 = 2 * TENT_LO - 1             # 49
RADIX = 2048.0                    # field packing radix (fields |.| <= 161)
PAD = 1000.0                      # > any real bin; exact in fp16
MAGIC = 8388608.0                 # 2^23
HALF = float(np.float32(0.49999997))
F32 = mybir.dt.float32
F16 = mybir.dt.float16
Op = mybir.AluOpType
Ax = mybir.AxisListType
ActF = None  # set lazily (mybir.ActivationFunctionType)

_NC_CACHE = {}
_HIST_OP_CACHE = {}


def _get_hist2_op():
    """Register (once per process) the custom DVE op
        out    = in1 * ([in0==s0] + imm2*[in0==s1])
        accum += sum(out)
    and return the DveOp handle."""
    if "op" in _HIST_OP_CACHE:
        return _HIST_OP_CACHE["op"]

    from concourse import dve_ops as dvo
    from concourse.dve_spec import Spec, Src0, Src1, C0, C1, C2, Zero, eq, lower
    from concourse.dve_uop import DveOpSpec

    name = "ECC_HIST2_ANT"

    def _ref(in0, in1, c0, c1, c2):
        a = in0.astype(np.float32)
        d = in1.astype(np.float32)
        body = (d * ((a == c0).astype(np.float32)
                     + c2 * (a == c1).astype(np.float32))).astype(np.float32)
        acc = body.reshape(body.shape[0], -1).sum(axis=-1, keepdims=True)
        return body, acc.astype(np.float32)

    spec = Spec(
        body=Src1 * (eq(Src0, C0) + C2 * eq(Src0, C1)),
        accum=_operator_add,
        accum_init=Zero,
        reference=_ref,
    )

    if name not in dvo._SUB_OPCODE_FOR_NAME:
        row = 1 + len(dvo.OPS)
        assert row < 0x20
        dvo._SUB_OPCODE_FOR_NAME[name] = row
        shas = {}
        for ver in ("v3", "v4"):
            sp = DveOpSpec(name=name, opcode=row, uops=lower(spec, ver=ver),
                           rd1_en=True)
            shas[ver] = sp.sha(ver)
        op = dvo.DveOp(name, spec, subdim=False, uops_sha=shas)
        dvo.OPS.append(op)
        dvo.CUSTOM_DVE_SPECS[name] = spec
    else:
        op = next(o for o in dvo.OPS if o.name == name)

    _HIST_OP_CACHE["op"] = op
    return op


def _build_nc():
    hist2 = _get_hist2_op()
    Act = mybir.ActivationFunctionType

    nc = bacc.Bacc(None, target_bir_lowering=False)
    x_in = nc.dram_tensor("x", [NCHUNK * 128, FD], F32, kind="ExternalInput")
    # bd: cols 0..71 = image selectors per chunk; cols 72.. = -c relu biases
    bd_in = nc.dram_tensor("bd", [128, NCHUNK * IMGS + NRC], F32,
                           kind="ExternalInput")
    out = nc.dram_tensor("out", [RBASE + NRC, IMGS], F32,
                         kind="ExternalOutput")

    with tile.TileContext(nc) as tc:
        with (
            tc.tile_pool(name="xp", bufs=2) as xp,
            tc.tile_pool(name="kp", bufs=1) as kp,
            tc.tile_pool(name="ap", bufs=1) as ap,
            tc.tile_pool(name="wp", bufs=1) as wp,
            tc.tile_pool(name="dp", bufs=1) as dp,
            tc.tile_pool(name="cst", bufs=1) as cst,
            tc.tile_pool(name="pp", bufs=1, space="PSUM") as pp,
        ):
            # warm-up: trigger Scalar's ACT_TABLE_LOAD at t=0
            warm = cst.tile([128, 1], F32)
            nc.gpsimd.memset(warm[:], 0.0)
            nc.scalar.activation(out=warm[:], in_=warm[:], func=Act.Copy,
                                 bias=0.0, scale=1.0)
            bdt = cst.tile([128, NCHUNK * IMGS + NRC], F32)
            nc.sync.dma_start(out=bdt[:], in_=bd_in[:])
            padt = cst.tile([CHUNK_IMGS, W], F16)
            nc.vector.memset(padt[:], PAD)
            # per-chunk decoded histogram [128, 26]:
            #   col 0 = tot (sum delta), cols 1..24 = DVE bins, col 25 pad
            hist = cst.tile([128, NH + 1], F32)
            nc.vector.memset(hist[:], 0.0)
            # per-chunk relu moment accumulators R_c, c = 49..62
            rt = cst.tile([128, NRC], F32)
            psum = pp.tile([NH, IMGS], F32)
            psum2 = pp.tile([NRC, IMGS], F32)

            def conv_and_halo(c, split, on_dve):
                """DMA chunk c, convert to f16 bins kt = 2k (rows 1..16 at
                cols W..W+FD).  Halo rows are loaded straight from DRAM and
                converted too; image-boundary halo rows get PAD via two tiny
                gpsimd SBUF copies.  Pipelined chunks (on_dve=False) convert
                on the Scalar engine."""
                kt = kp.tile([128, (ROWS + 2) * W], F16, tag=f"kt{c}")
                xt = xp.tile([128, FD], F32, tag="xt")
                xh = xp.tile([128, 2 * W], F32, tag="xh")

                # pipelined chunks convert on the (idle) GpSimd engine so the
                # Scalar engine's whole window is free for relu moments
                ceng = nc.vector if on_dve else nc.gpsimd

                def affine(out_, in_):
                    ceng.tensor_scalar(
                        out=out_, in0=in_, scalar1=31.0, scalar2=HALF,
                        op0=Op.mult, op1=Op.add)

                def roundto(out_, in_):
                    # round(y) then double: kt = 2*round(y)
                    ceng.tensor_scalar(
                        out=in_, in0=in_, scalar1=MAGIC, scalar2=0.0,
                        op0=Op.add, op1=Op.add)
                    ceng.tensor_scalar(
                        out=out_, in0=in_, scalar1=-MAGIC, scalar2=2.0,
                        op0=Op.add, op1=Op.mult)

                # main rows first; halo reads are strided and slow, keep them
                # off the main DMA's critical path
                nfd = FD // split
                for h in range(split):
                    sl = slice(h * nfd, (h + 1) * nfd)
                    nc.sync.dma_start(
                        out=xt[:, sl], in_=x_in[c * 128:(c + 1) * 128, sl])
                for h in range(split):
                    sl = slice(h * nfd, (h + 1) * nfd)
                    ksl = slice(W + h * nfd, W + (h + 1) * nfd)
                    affine(xt[:, sl], xt[:, sl])
                    roundto(kt[:, ksl], xt[:, sl])
                # halo sources: top halo of p = last row of p-1, bottom halo
                # of p = first row of p+1 (partition-shifted DRAM reads).
                if c == 0:
                    nc.sync.dma_start(
                        out=xh[0:1, 0:W], in_=x_in[0:1, FD - W:FD])
                    nc.sync.dma_start(
                        out=xh[1:128, 0:W], in_=x_in[0:127, FD - W:FD])
                else:
                    nc.sync.dma_start(
                        out=xh[:, 0:W],
                        in_=x_in[c * 128 - 1:c * 128 + 127, FD - W:FD])
                if c == NCHUNK - 1:
                    nc.sync.dma_start(
                        out=xh[0:127, W:2 * W],
                        in_=x_in[c * 128 + 1:c * 128 + 128, 0:W])
                    nc.sync.dma_start(
                        out=xh[127:128, W:2 * W],
                        in_=x_in[c * 128 + 127:c * 128 + 128, 0:W])
                else:
                    nc.sync.dma_start(
                        out=xh[:, W:2 * W],
                        in_=x_in[c * 128 + 1:c * 128 + 129, 0:W])
                # halo conversion (full width; edge rows are dummies)
                affine(xh[:, :], xh[:, :])
                roundto(kt[:, 0:W], xh[:, 0:W])
                roundto(kt[:, FD + W:FD + 2 * W], xh[:, W:2 * W])
                # image-boundary halo rows: PAD
                ktop = kt[:, 0:W].rearrange("(a b) w -> a b w", b=RB)
                nc.gpsimd.dma_start(out=ktop[:, 0, :], in_=padt[:])
                kbot = kt[:, FD + W:FD + 2 * W].rearrange("(a b) w -> a b w", b=RB)
                nc.gpsimd.dma_start(out=kbot[:, RB - 1, :], in_=padt[:])
                return kt

            def pre_assembly(kt, eng):
                """The four kt-neighborhood compares."""
                # rh[r, j] = [k(r, j+1) >= k(r, j)], own rows, j = 0..254
                # (col 255 crosses rows; harmless, later masked via t zeroing).
                # Stops one col short of the bottom halo (no halo dep); the
                # final col is zeroed (it is multiplied by cc = 0 downstream).
                rh = ap.tile([128, FD], F16, tag="rh")
                eng.memset(rh[:, FD - 1:FD], 0.0)
                eng.tensor_tensor(
                    out=rh[:, 0:FD - 1], in0=kt[:, W + 1:W + FD],
                    in1=kt[:, W:W + FD - 1], op=Op.is_ge)
                # rv[t, j] = [k(row t+1) >= k(row t)], t = 0..16 (17 rows)
                rv = ap.tile([128, (ROWS + 1) * W], F16, tag="rv")
                eng.tensor_tensor(
                    out=rv[:], in0=kt[:, W:], in1=kt[:, 0:(ROWS + 1) * W],
                    op=Op.is_ge)
                # khe[r, j] = max(k(r, j), k(r, j+1)), rows 0..17
                khe = ap.tile([128, (ROWS + 2) * W], F16, tag="khe")
                eng.tensor_tensor(
                    out=khe[:, 0:(ROWS + 2) * W - 1],
                    in0=kt[:, 0:(ROWS + 2) * W - 1], in1=kt[:, 1:(ROWS + 2) * W],
                    op=Op.max)
                eng.memset(khe[:, (ROWS + 2) * W - 1:(ROWS + 2) * W], PAD)
                # u[t, j] = [khe(row t+1, j) >= khe(row t, j)], t = 0..16
                ut = ap.tile([128, (ROWS + 1) * W], F16, tag="ut")
                eng.tensor_tensor(
                    out=ut[:], in0=khe[:, W:], in1=khe[:, 0:(ROWS + 1) * W],
                    op=Op.is_ge)
                return rh, rv, ut

            kts = [None] * NCHUNK
            pre = [None] * NCHUNK
            dls = [None] * NCHUNK
            kms = [None] * NCHUNK

            def delta_km(c):
                """Delta assembly + km for chunk c (Vector), tot on Scalar."""
                kt = kts[c]
                rh, rv, ut = pre[c]
                # Cc[r, j] = u(r) - u(r-1) for own rows r
                cc = wp.tile([128, FD], F16, tag="cc")
                nc.vector.tensor_tensor(
                    out=cc[:], in0=ut[:, W:], in1=ut[:, 0:FD], op=Op.subtract)
                # zero col 255 of each row (cross-row garbage in rh/cc)
                cc3 = cc[:].rearrange("p (r w) -> p r w", w=W)
                nc.vector.memset(cc3[:, :, W - 1:W], 0.0)
                # t = rh * Cc
                tt = wp.tile([128, FD], F16, tag="tt")
                nc.vector.tensor_tensor(out=tt[:], in0=rh[:], in1=cc[:],
                                        op=Op.mult)
                # delta = rv(below) - rv(above) + t - shift1(t) - Cc
                # (dl/km get distinct per-chunk buffers, kept live to the end
                # of the program so the arena never reuses their space for a
                # later chunk's kt, whose GpSimd writes would race the late
                # cross-engine readers)
                dl = dp.tile([128, FD], F16, tag=f"dl{c}")
                nc.vector.tensor_tensor(
                    out=dl[:], in0=rv[:, W:], in1=rv[:, 0:FD], op=Op.subtract)
                nc.vector.tensor_tensor(out=dl[:], in0=dl[:], in1=tt[:],
                                        op=Op.add)
                nc.vector.tensor_tensor(
                    out=dl[:, 1:FD], in0=dl[:, 1:FD], in1=tt[:, 0:FD - 1],
                    op=Op.subtract)
                nc.vector.tensor_tensor(
                    out=dl[:], in0=dl[:], in1=cc[:], op=Op.subtract)
                # tot = sum(delta) via the Scalar activation accumulator,
                # straight into hist col 0
                wmb = wp.tile([128, FD], F16, tag="wmb")
                nc.scalar.activation(
                    out=wmb[:], in_=dl[:], func=Act.Copy,
                    bias=0.0, scale=1.0, accum_out=hist[:, 0:1])
                # km = kt + delta/16 (cluster b sits at 2b + delta/16; tents
                # sampled at half-integer offsets recover fields exactly)
                km = dp.tile([128, FD], F16, tag=f"km{c}")
                nc.vector.scalar_tensor_tensor(
                    out=km[:], in0=dl[:], scalar=1.0 / 16.0,
                    in1=kt[:, W:W + FD], op0=Op.mult, op1=Op.add)
                return dl, km

            kts[0] = conv_and_halo(0, split=4, on_dve=True)
            pre[0] = pre_assembly(kts[0], nc.vector)
            dls[0], kms[0] = delta_km(0)

            for c in range(NCHUNK):
                kt = kts[c]
                dl, km = dls[c], kms[c]
                wmb = wp.tile([128, FD], F16, tag="wmb")

                # next chunk's DMA + Scalar conversion go first on the Scalar
                # queue; this chunk's relu moments fill the remaining window
                if c + 1 < NCHUNK:
                    kts[c + 1] = conv_and_halo(c + 1, split=1, on_dve=False)

                # --- Scalar relu moments: R_c = sum relu(km - c) ---
                for j in range(NRC):
                    nc.scalar.activation(
                        out=wmb[:], in_=km[:], func=Act.Relu,
                        bias=bdt[:, NCHUNK * IMGS + j:NCHUNK * IMGS + j + 1],
                        scale=1.0, accum_out=rt[:, j:j + 1])

                # --- 12 packed-histogram passes on the Vector engine, with
                # the next chunk's delta assembly interleaved mid-stream so
                # its relu moments (and the kernel tail) can start early ---
                acc = wp.tile([128, NPAIR], F32, tag="acc")
                wm = wp.tile([128, FD], F16, tag="wm")
                for g in range(NPAIR):
                    nc.vector._custom_dve(
                        hist2,
                        out=wm[:],
                        in0=kt[:, W:W + FD],
                        in1=dl[:],
                        s0=float(2 * (2 * g + 1)),
                        s1=float(2 * (2 * g + 2)),
                        imm2=RADIX,
                        accum_out=acc[:, g:g + 1],
                    )
                    if g == 6 and c + 1 < NCHUNK:
                        pre[c + 1] = pre_assembly(kts[c + 1], nc.vector)
                        dls[c + 1], kms[c + 1] = delta_km(c + 1)

                # --- decode packed fields into hist[:, 1..24] ---
                # hi = round(acc / RADIX); lo = acc - RADIX*hi
                dec = wp.tile([128, NPAIR], F32, tag="dec")
                nc.vector.tensor_scalar(
                    out=dec[:], in0=acc[:], scalar1=1.0 / RADIX, scalar2=MAGIC,
                    op0=Op.mult, op1=Op.add)
                # hi -> even bins 2,4,...,24
                hist_hi = hist[:, 2:DVEBINS + 2].rearrange(
                    "p (g two) -> p g two", two=2)
                nc.vector.tensor_scalar(
                    out=hist_hi[:, :, 0], in0=dec[:], scalar1=-MAGIC, scalar2=0.0,
                    op0=Op.add, op1=Op.add)
                # lo = acc - RADIX*hi -> odd bins 1,3,...,23
                hist_lo = hist[:, 1:DVEBINS + 1].rearrange(
                    "p (g two) -> p g two", two=2)
                nc.vector.scalar_tensor_tensor(
                    out=hist_lo[:, :, 0], in0=hist_hi[:, :, 0], scalar=-RADIX,
                    in1=acc[:], op0=Op.mult, op1=Op.add)

                # --- partition partials -> per-image (PSUM accumulate) ---
                nc.tensor.matmul(
                    psum[:], hist[:, 0:NH],
                    bdt[:, c * IMGS:(c + 1) * IMGS],
                    start=(c == 0), stop=(c == NCHUNK - 1))
                nc.tensor.matmul(
                    psum2[:], rt[:],
                    bdt[:, c * IMGS:(c + 1) * IMGS],
                    start=(c == 0), stop=(c == NCHUNK - 1))

            # liveness pins: tiny end-of-program reads keep every chunk's
            # dl/km buffer allocated for the whole kernel (see note above)
            keep = cst.tile([128, 3 * NCHUNK], F16)
            for i, t in enumerate(dls + kms + kts):
                nc.vector.tensor_copy(out=keep[:, i:i + 1], in_=t[:, 0:1])

            # per-image partials to DRAM; decode + cumsum happen on the host
            houtt = cst.tile([RBASE + NRC, IMGS], F32)
            nc.vector.tensor_copy(out=houtt[0:NH, :], in_=psum[:])
            nc.vector.tensor_copy(out=houtt[RBASE:RBASE + NRC, :],
                                  in_=psum2[:])
            nc.sync.dma_start(out=out[0:NH, :], in_=houtt[0:NH, :])
            nc.sync.dma_start(out=out[RBASE:RBASE + NRC, :],
                              in_=houtt[RBASE:RBASE + NRC, :])

    nc.finalize()
    return nc


def _post(raw):
    """[RBASE+NRC, IMGS] per-image partials -> [IMGS/C, C, STEPS] ECC curves.

    raw rows: 0 = tot, 1..24 = F_b, 32..45 = R_c (c = 49..62).
    Tent decode: T_b = R[2b]-2R[2b+1]+R[2b+2], U_b = R[2b-1]-2R[2b]+R[2b+1],
    F_b = 8*(T_b - U_b) for b = 25..30;  F_31 = tot - sum(F_1..30).
    """
    F = np.zeros((STEPS, IMGS), dtype=np.float64)          # F[b] for b=0..31
    F[1:DVEBINS + 1] = raw[1:DVEBINS + 1]
    R = raw[RBASE:RBASE + NRC].astype(np.float64)          # R[c-RC0]
    for b in range(TENT_LO, TENT_HI + 1):
        i = 2 * b - RC0
        U = R[i - 1] - 2.0 * R[i] + R[i + 1]
        T = R[i] - 2.0 * R[i + 1] + R[i + 2]
        F[b] = 8.0 * (T - U)
    F[STEPS - 1] = raw[0] - F[1:STEPS - 1].sum(axis=0)
    ecc = np.cumsum(F, axis=0)                             # [32, 24]
    return ecc.T.reshape(B // NCORES, C, STEPS).astype(np.float32)


def _bd_host():
    bd = np.zeros((128, NCHUNK * IMGS + NRC), dtype=np.float32)
    for c in range(NCHUNK):
        for p in range(128):
            bd[p, c * IMGS + c * CHUNK_IMGS + p // RB] = 1.0
    for j in range(NRC):
        bd[:, NCHUNK * IMGS + j] = -(float(RC0) - 0.5 + j)
    return bd


def kernel(x: np.ndarray) -> np.ndarray:
    assert x.shape == (B, C, H, W) and x.dtype == np.float32
    if "nc" not in _NC_CACHE:
        _NC_CACHE["nc"] = _build_nc()
    nc = _NC_CACHE["nc"]

    bd = _bd_host()
    in_maps = []
    for i in range(NCORES):
        shard = x[i * (B // NCORES):(i + 1) * (B // NCORES)]  # (8, 3, 256, 256)
        in_maps.append({
            "x": np.ascontiguousarray(shard).reshape(NCHUNK * 128, FD),
            "bd": bd,
        })
    res = run_bass_kernel_spmd(nc, in_maps, core_ids=list(range(NCORES)))
    parts = [_post(res.results[i]["out"]) for i in range(NCORES)]
    return np.concatenate(parts, axis=0).reshape(B, C * STEPS).astype(np.float32)


if __name__ == "__main__":
    rng = np.random.default_rng(0)
    x = rng.random((B, C, H, W), dtype=np.float32)
    y = kernel(x)
    print("kernel out", y.shape, y.dtype, y[:2, :6])


# revision 52
# speedup vs baseline: 1.5304x; 1.5304x over previous
"""Trainium2 Bass kernel for nn_CubECLayr: Euler characteristic curves of
sublevel cubical complexes, batch-data-parallel over 8 NeuronCores.

Algorithm (per core, 24 images of 256x256):
  1. kt = 2*ceil(x/DT) per pixel (exact even-integer bin, via fused multiply
     + magic-number round).  kt in [2, 62] for x in (0,1).
  2. Vertex attribution: every cell (vertex/edge/square) of the cubical
     complex is anchored to its (value, index)-max vertex; the signed count
     of cells anchored at each pixel is an integer delta computed from
     neighbor comparisons in kt-space.  Then ECC_s = sum_p delta_p*[k_p <= s].
  3. Per-bin fields F_b = sum_p delta_p*[k_p == b] are computed two ways,
     split across engines:
       - bins 1..24 on the Vector engine: a custom DVE op accumulates TWO
         radix-packed bins per pass (accum += delta*([kt==2b0]+2048*[kt==2b1]),
         12 passes; fields bounded by 161 << 1024 so fp32 decodes exactly).
       - bins 25..30 on the otherwise-idle Scalar engine via relu moments:
         km = kt + (delta+8)/16;  R_c = sum_p relu(km - c) for c = 49..62
         (14 activation passes with accumulator).  Host-side second
         differences give tents T_b = R_{2b}-2R_{2b+1}+R_{2b+2} and
         U_b = R_{2b-1}-2R_{2b}+R_{2b+1} with F_b = 8*(T_b - U_b), exact in
         fp32 (all values are multiples of 1/16 bounded by 2^23).
     bin 31 = tot - sum(bins 1..30) on host, where tot = sum(delta) rides
     the accumulator of a Scalar copy of delta.
  4. Per-partition partials reduce to per-image values by two small PE
     matmuls into one PSUM tile ([33+14, 24]); host does the final cumsum.

Pipelining: kt is double-buffered; chunk c+1's DMA + Scalar bin-conversion
is issued between chunk c's delta assembly and its histogram passes.  The
Scalar relu passes for chunk c run concurrently with chunk c's (and early
chunk c+1's) Vector histogram passes.

Layout: 3 chunks x 8 images; each image owns 16 partitions (16 rows each,
one halo row above/below; image-boundary halo rows are PAD).
"""

from operator import add as _operator_add

import numpy as np

import concourse.bacc as bacc
import concourse.mybir as mybir
from concourse import tile
from concourse.bass_utils import run_bass_kernel_spmd

NCORES = 8
B, C, H, W = 64, 3, 256, 256
IMGS = (B // NCORES) * C          # 24 images per core
CHUNK_IMGS = 8
NCHUNK = IMGS // CHUNK_IMGS       # 3
RB = 16                           # partitions per image
ROWS = H // RB                    # 16 own rows per partition
FD = ROWS * W                     # 4096 own pixels per partition
STEPS = 32
DVEBINS = 24                      # bins 1..24 via custom DVE passes
NH = DVEBINS + 1                  # hist cols: 0 = tot, 1..24 = DVE bins
RBASE = 32                        # R rows base partition in PSUM (HW: 0/32/64)
NPAIR = DVEBINS // 2              # 12 packed passes
TENT_LO = DVEBINS + 1             # bins 25..30 via Scalar relu moments
TENT_HI = 30
NRC = 2 * TENT_HI + 2 - (2 * TENT_LO - 1) + 1   # R_c count: c = 49..62
RC0 = 2 * TENT_LO - 1             # 49
RADIX = 2048.0                    # field packing radix (fields |.| <= 161)
PAD = 1000.0                      # > any real bin; exact in fp16
MAGIC = 8388608.0                 # 2^23
HALF = float(np.float32(0.49999997))
F32 = mybir.dt.float32
F16 = mybir.dt.float16
Op = mybir.AluOpType
Ax = mybir.AxisListType
ActF = None  # set lazily (mybir.ActivationFunctionType)

_NC_CACHE = {}
_HIST_OP_CACHE = {}


def _get_hist2_op():
    """Register (once per process) the custom DVE op
        out    = in1 * ([in0==s0] + imm2*[in0==s1])
        accum += sum(out)
    and return the DveOp handle."""
    if "op" in _HIST_OP_CACHE:
        return _HIST_OP_CACHE["op"]

    from concourse import dve_ops as dvo
    from concourse.dve_spec import Spec, Src0, Src1, C0, C1, C2, Zero, eq, lower
    from concourse.dve_uop import DveOpSpec

    name = "ECC_HIST2_ANT"

    def _ref(in0, in1, c0, c1, c2):
        a = in0.astype(np.float32)
        d = in1.astype(np.float32)
        body = (d * ((a == c0).astype(np.float32)
                     + c2 * (a == c1).astype(np.float32))).astype(np.float32)
        acc = body.reshape(body.shape[0], -1).sum(axis=-1, keepdims=True)
        return body, acc.astype(np.float32)

    spec = Spec(
        body=Src1 * (eq(Src0, C0) + C2 * eq(Src0, C1)),
        accum=_operator_add,
        accum_init=Zero,
        reference=_ref,
    )

    if name not in dvo._SUB_OPCODE_FOR_NAME:
        row = 1 + len(dvo.OPS)
        assert row < 0x20
        dvo._SUB_OPCODE_FOR_NAME[name] = row
        shas = {}
        for ver in ("v3", "v4"):
            sp = DveOpSpec(name=name, opcode=row, uops=lower(spec, ver=ver),
                           rd1_en=True)
            shas[ver] = sp.sha(ver)
        op = dvo.DveOp(name, spec, subdim=False, uops_sha=shas)
        dvo.OPS.append(op)
        dvo.CUSTOM_DVE_SPECS[name] = spec
    else:
        op = next(o for o in dvo.OPS if o.name == name)

    _HIST_OP_CACHE["op"] = op
    return op


def _build_nc():
    hist2 = _get_hist2_op()
    Act = mybir.ActivationFunctionType

    nc = bacc.Bacc(None, target_bir_lowering=False)
    x_in = nc.dram_tensor("x", [NCHUNK * 128, FD], F32, kind="ExternalInput")
    # bd: cols 0..71 = image selectors per chunk; cols 72.. = -c relu biases
    bd_in = nc.dram_tensor("bd", [128, NCHUNK * IMGS + NRC], F32,
                           kind="ExternalInput")
    out = nc.dram_tensor("out", [RBASE + NRC, IMGS], F32,
                         kind="ExternalOutput")

    with tile.TileContext(nc) as tc:
        with (
            tc.tile_pool(name="xp", bufs=2) as xp,
            tc.tile_pool(name="kp", bufs=2) as kp,
            tc.tile_pool(name="ap", bufs=2) as ap,
            tc.tile_pool(name="wp", bufs=1) as wp,
            tc.tile_pool(name="dp", bufs=3) as dp,
            tc.tile_pool(name="cst", bufs=1) as cst,
            tc.tile_pool(name="pp", bufs=1, space="PSUM") as pp,
        ):
            # warm-up: trigger Scalar's ACT_TABLE_LOAD at t=0
            warm = cst.tile([128, 1], F32)
            nc.gpsimd.memset(warm[:], 0.0)
            nc.scalar.activation(out=warm[:], in_=warm[:], func=Act.Copy,
                                 bias=0.0, scale=1.0)
            bdt = cst.tile([128, NCHUNK * IMGS + NRC], F32)
            nc.sync.dma_start(out=bdt[:], in_=bd_in[:])
            padt = cst.tile([CHUNK_IMGS, W], F16)
            nc.vector.memset(padt[:], PAD)
            # per-chunk decoded histogram [128, 26]:
            #   col 0 = tot (sum delta), cols 1..24 = DVE bins, col 25 pad
            hist = cst.tile([128, NH + 1], F32)
            nc.vector.memset(hist[:], 0.0)
            # per-chunk relu moment accumulators R_c, c = 49..62
            rt = cst.tile([128, NRC], F32)
            psum = pp.tile([NH, IMGS], F32)
            psum2 = pp.tile([NRC, IMGS], F32)

            def conv_and_halo(c, split, on_dve):
                """DMA chunk c, convert to f16 bins kt = 2k (rows 1..16 at
                cols W..W+FD).  Halo rows are loaded straight from DRAM and
                converted too; image-boundary halo rows get PAD via two tiny
                gpsimd SBUF copies.  Pipelined chunks (on_dve=False) convert
                on the Scalar engine."""
                kt = kp.tile([128, (ROWS + 2) * W], F16, tag="kt")
                xt = xp.tile([128, FD], F32, tag="xt")
                xh = xp.tile([128, 2 * W], F32, tag="xh")

                def affine(out_, in_):
                    if on_dve:
                        nc.vector.tensor_scalar(
                            out=out_, in0=in_, scalar1=31.0, scalar2=HALF,
                            op0=Op.mult, op1=Op.add)
                    else:
                        nc.scalar.activation(out=out_, in_=in_, func=Act.Copy,
                                             bias=HALF, scale=31.0)

                def roundto(out_, in_):
                    # round(y) then double: kt = 2*round(y)
                    if on_dve:
                        nc.vector.tensor_scalar(
                            out=in_, in0=in_, scalar1=MAGIC, scalar2=0.0,
                            op0=Op.add, op1=Op.add)
                        nc.vector.tensor_scalar(
                            out=out_, in0=in_, scalar1=-MAGIC, scalar2=2.0,
                            op0=Op.add, op1=Op.mult)
                    else:
                        nc.scalar.activation(out=in_, in_=in_, func=Act.Copy,
                                             bias=MAGIC, scale=1.0)
                        nc.scalar.activation(out=out_, in_=in_, func=Act.Copy,
                                             bias=-2.0 * MAGIC, scale=2.0)

                # main rows first; halo reads are strided and slow, keep them
                # off the main DMA's critical path
                nfd = FD // split
                for h in range(split):
                    sl = slice(h * nfd, (h + 1) * nfd)
                    nc.sync.dma_start(
                        out=xt[:, sl], in_=x_in[c * 128:(c + 1) * 128, sl])
                for h in range(split):
                    sl = slice(h * nfd, (h + 1) * nfd)
                    ksl = slice(W + h * nfd, W + (h + 1) * nfd)
                    affine(xt[:, sl], xt[:, sl])
                    roundto(kt[:, ksl], xt[:, sl])
                # halo sources: top halo of p = last row of p-1, bottom halo
                # of p = first row of p+1 (partition-shifted DRAM reads).
                if c == 0:
                    nc.sync.dma_start(
                        out=xh[0:1, 0:W], in_=x_in[0:1, FD - W:FD])
                    nc.sync.dma_start(
                        out=xh[1:128, 0:W], in_=x_in[0:127, FD - W:FD])
                else:
                    nc.sync.dma_start(
                        out=xh[:, 0:W],
                        in_=x_in[c * 128 - 1:c * 128 + 127, FD - W:FD])
                if c == NCHUNK - 1:
                    nc.sync.dma_start(
                        out=xh[0:127, W:2 * W],
                        in_=x_in[c * 128 + 1:c * 128 + 128, 0:W])
                    nc.sync.dma_start(
                        out=xh[127:128, W:2 * W],
                        in_=x_in[c * 128 + 127:c * 128 + 128, 0:W])
                else:
                    nc.sync.dma_start(
                        out=xh[:, W:2 * W],
                        in_=x_in[c * 128 + 1:c * 128 + 129, 0:W])
                # halo conversion (full width; edge rows are dummies)
                affine(xh[:, :], xh[:, :])
                roundto(kt[:, 0:W], xh[:, 0:W])
                roundto(kt[:, FD + W:FD + 2 * W], xh[:, W:2 * W])
                # image-boundary halo rows: PAD
                ktop = kt[:, 0:W].rearrange("(a b) w -> a b w", b=RB)
                nc.gpsimd.dma_start(out=ktop[:, 0, :], in_=padt[:])
                kbot = kt[:, FD + W:FD + 2 * W].rearrange("(a b) w -> a b w", b=RB)
                nc.gpsimd.dma_start(out=kbot[:, RB - 1, :], in_=padt[:])
                return kt

            def pre_assembly(kt, eng):
                """The four kt-neighborhood compares."""
                # rh[r, j] = [k(r, j+1) >= k(r, j)], own rows, j = 0..254
                # (col 255 crosses rows; harmless, later masked via t zeroing).
                # Stops one col short of the bottom halo (no halo dep); the
                # final col is zeroed (it is multiplied by cc = 0 downstream).
                rh = ap.tile([128, FD], F16, tag="rh")
                eng.memset(rh[:, FD - 1:FD], 0.0)
                eng.tensor_tensor(
                    out=rh[:, 0:FD - 1], in0=kt[:, W + 1:W + FD],
                    in1=kt[:, W:W + FD - 1], op=Op.is_ge)
                # rv[t, j] = [k(row t+1) >= k(row t)], t = 0..16 (17 rows)
                rv = ap.tile([128, (ROWS + 1) * W], F16, tag="rv")
                eng.tensor_tensor(
                    out=rv[:], in0=kt[:, W:], in1=kt[:, 0:(ROWS + 1) * W],
                    op=Op.is_ge)
                # khe[r, j] = max(k(r, j), k(r, j+1)), rows 0..17
                khe = ap.tile([128, (ROWS + 2) * W], F16, tag="khe")
                eng.tensor_tensor(
                    out=khe[:, 0:(ROWS + 2) * W - 1],
                    in0=kt[:, 0:(ROWS + 2) * W - 1], in1=kt[:, 1:(ROWS + 2) * W],
                    op=Op.max)
                eng.memset(khe[:, (ROWS + 2) * W - 1:(ROWS + 2) * W], PAD)
                # u[t, j] = [khe(row t+1, j) >= khe(row t, j)], t = 0..16
                ut = ap.tile([128, (ROWS + 1) * W], F16, tag="ut")
                eng.tensor_tensor(
                    out=ut[:], in0=khe[:, W:], in1=khe[:, 0:(ROWS + 1) * W],
                    op=Op.is_ge)
                return rh, rv, ut

            kts = [None] * NCHUNK
            pre = [None] * NCHUNK
            dls = [None] * NCHUNK
            kms = [None] * NCHUNK

            def delta_km(c):
                """Delta assembly + km for chunk c (Vector), tot on Scalar."""
                kt = kts[c]
                rh, rv, ut = pre[c]
                # Cc[r, j] = u(r) - u(r-1) for own rows r
                cc = wp.tile([128, FD], F16, tag="cc")
                nc.vector.tensor_tensor(
                    out=cc[:], in0=ut[:, W:], in1=ut[:, 0:FD], op=Op.subtract)
                # zero col 255 of each row (cross-row garbage in rh/cc)
                cc3 = cc[:].rearrange("p (r w) -> p r w", w=W)
                nc.vector.memset(cc3[:, :, W - 1:W], 0.0)
                # t = rh * Cc
                tt = wp.tile([128, FD], F16, tag="tt")
                nc.vector.tensor_tensor(out=tt[:], in0=rh[:], in1=cc[:],
                                        op=Op.mult)
                # delta = rv(below) - rv(above) + t - shift1(t) - Cc
                dl = dp.tile([128, FD], F16, tag="dl")
                nc.vector.tensor_tensor(
                    out=dl[:], in0=rv[:, W:], in1=rv[:, 0:FD], op=Op.subtract)
                nc.vector.tensor_tensor(out=dl[:], in0=dl[:], in1=tt[:],
                                        op=Op.add)
                nc.vector.tensor_tensor(
                    out=dl[:, 1:FD], in0=dl[:, 1:FD], in1=tt[:, 0:FD - 1],
                    op=Op.subtract)
                nc.vector.tensor_tensor(
                    out=dl[:], in0=dl[:], in1=cc[:], op=Op.subtract)
                # tot = sum(delta) via the Scalar activation accumulator,
                # straight into hist col 0
                wmb = wp.tile([128, FD], F16, tag="wmb")
                nc.scalar.activation(
                    out=wmb[:], in_=dl[:], func=Act.Copy,
                    bias=0.0, scale=1.0, accum_out=hist[:, 0:1])
                # km = kt + delta/16 (cluster b sits at 2b + delta/16; tents
                # sampled at half-integer offsets recover fields exactly)
                km = dp.tile([128, FD], F16, tag="km")
                nc.vector.scalar_tensor_tensor(
                    out=km[:], in0=dl[:], scalar=1.0 / 16.0,
                    in1=kt[:, W:W + FD], op0=Op.mult, op1=Op.add)
                return dl, km

            kts[0] = conv_and_halo(0, split=4, on_dve=True)
            pre[0] = pre_assembly(kts[0], nc.vector)
            dls[0], kms[0] = delta_km(0)

            for c in range(NCHUNK):
                kt = kts[c]
                dl, km = dls[c], kms[c]
                wmb = wp.tile([128, FD], F16, tag="wmb")

                # next chunk's DMA + Scalar conversion go first on the Scalar
                # queue; this chunk's relu moments fill the remaining window
                if c + 1 < NCHUNK:
                    kts[c + 1] = conv_and_halo(c + 1, split=1, on_dve=False)

                # --- Scalar relu moments: R_c = sum relu(km - c) ---
                for j in range(NRC):
                    nc.scalar.activation(
                        out=wmb[:], in_=km[:], func=Act.Relu,
                        bias=bdt[:, NCHUNK * IMGS + j:NCHUNK * IMGS + j + 1],
                        scale=1.0, accum_out=rt[:, j:j + 1])

                # --- 12 packed-histogram passes on the Vector engine, with
                # the next chunk's delta assembly interleaved mid-stream so
                # its relu moments (and the kernel tail) can start early ---
                acc = wp.tile([128, NPAIR], F32, tag="acc")
                wm = wp.tile([128, FD], F16, tag="wm")
                for g in range(NPAIR):
                    nc.vector._custom_dve(
                        hist2,
                        out=wm[:],
                        in0=kt[:, W:W + FD],
                        in1=dl[:],
                        s0=float(2 * (2 * g + 1)),
                        s1=float(2 * (2 * g + 2)),
                        imm2=RADIX,
                        accum_out=acc[:, g:g + 1],
                    )
                    if g == 6 and c + 1 < NCHUNK:
                        pre[c + 1] = pre_assembly(kts[c + 1], nc.vector)
                        dls[c + 1], kms[c + 1] = delta_km(c + 1)

                # --- decode packed fields into hist[:, 1..24] ---
                # hi = round(acc / RADIX); lo = acc - RADIX*hi
                dec = wp.tile([128, NPAIR], F32, tag="dec")
                nc.vector.tensor_scalar(
                    out=dec[:], in0=acc[:], scalar1=1.0 / RADIX, scalar2=MAGIC,
                    op0=Op.mult, op1=Op.add)
                # hi -> even bins 2,4,...,24
                hist_hi = hist[:, 2:DVEBINS + 2].rearrange(
                    "p (g two) -> p g two", two=2)
                nc.vector.tensor_scalar(
                    out=hist_hi[:, :, 0], in0=dec[:], scalar1=-MAGIC, scalar2=0.0,
                    op0=Op.add, op1=Op.add)
                # lo = acc - RADIX*hi -> odd bins 1,3,...,23
                hist_lo = hist[:, 1:DVEBINS + 1].rearrange(
                    "p (g two) -> p g two", two=2)
                nc.vector.scalar_tensor_tensor(
                    out=hist_lo[:, :, 0], in0=hist_hi[:, :, 0], scalar=-RADIX,
                    in1=acc[:], op0=Op.mult, op1=Op.add)

                # --- partition partials -> per-image (PSUM accumulate) ---
                nc.tensor.matmul(
                    psum[:], hist[:, 0:NH],
                    bdt[:, c * IMGS:(c + 1) * IMGS],
                    start=(c == 0), stop=(c == NCHUNK - 1))
                nc.tensor.matmul(
                    psum2[:], rt[:],
                    bdt[:, c * IMGS:(c + 1) * IMGS],
                    start=(c == 0), stop=(c == NCHUNK - 1))

            # per-image partials to DRAM; decode + cumsum happen on the host
            houtt = cst.tile([RBASE + NRC, IMGS], F32)
            nc.vector.tensor_copy(out=houtt[0:NH, :], in_=psum[:])
            nc.vector.tensor_copy(out=houtt[RBASE:RBASE + NRC, :],
                                  in_=psum2[:])
            nc.sync.dma_start(out=out[0:NH, :], in_=houtt[0:NH, :])
            nc.sync.dma_start(out=out[RBASE:RBASE + NRC, :],
                              in_=houtt[RBASE:RBASE + NRC, :])

    nc.finalize()
    return nc


def _post(raw):
    """[RBASE+NRC, IMGS] per-image partials -> [IMGS/C, C, STEPS] ECC curves.

    raw rows: 0 = tot, 1..24 = F_b, 32..45 = R_c (c = 49..62).
    Tent decode: T_b = R[2b]-2R[2b+1]+R[2b+2], U_b = R[2b-1]-2R[2b]+R[2b+1],
    F_b = 8*(T_b - U_b) for b = 25..30;  F_31 = tot - sum(F_1..30).
    """
    F = np.zeros((STEPS, IMGS), dtype=np.float64)          # F[b] for b=0..31
    F[1:DVEBINS + 1] = raw[1:DVEBINS + 1]
    R = raw[RBASE:RBASE + NRC].astype(np.float64)          # R[c-RC0]
    for b in range(TENT_LO, TENT_HI + 1):
        i = 2 * b - RC0
        U = R[i - 1] - 2.0 * R[i] + R[i + 1]
        T = R[i] - 2.0 * R[i + 1] + R[i + 2]
        F[b] = 8.0 * (T - U)
    F[STEPS - 1] = raw[0] - F[1:STEPS - 1].sum(axis=0)
    ecc = np.cumsum(F, axis=0)                             # [32, 24]
    return ecc.T.reshape(B // NCORES, C, STEPS).astype(np.float32)


def _bd_host():
    bd = np.zeros((128, NCHUNK * IMGS + NRC), dtype=np.float32)
    for c in range(NCHUNK):
        for p in range(128):
            bd[p, c * IMGS + c * CHUNK_IMGS + p // RB] = 1.0
    for j in range(NRC):
        bd[:, NCHUNK * IMGS + j] = -(float(RC0) - 0.5 + j)
    return bd


def kernel(x: np.ndarray) -> np.ndarray:
    assert x.shape == (B, C, H, W) and x.dtype == np.float32
    if "nc" not in _NC_CACHE:
        _NC_CACHE["nc"] = _build_nc()
    nc = _NC_CACHE["nc"]

    bd = _bd_host()
    in_maps = []
    for i in range(NCORES):
        shard = x[i * (B // NCORES):(i + 1) * (B // NCORES)]  # (8, 3, 256, 256)
        in_maps.append({
            "x": np.ascontiguousarray(shard).reshape(NCHUNK * 128, FD),
            "bd": bd,
        })
    res = run_bass_kernel_spmd(nc, in_maps, core_ids=list(range(NCORES)))
    parts = [_post(res.results[i]["out"]) for i in range(NCORES)]
    return np.concatenate(parts, axis=0).reshape(B, C * STEPS).astype(np.float32)


if __name__ == "__main__":
    rng = np.random.default_rng(0)
    x = rng.random((B, C, H, W), dtype=np.float32)
    y = kernel(x)
    print("kernel out", y.shape, y.dtype, y[:2, :6])
